# revision 1
# baseline (speedup 1.0000x reference)
"""Trainium2 Bass kernel for nn_L1AttnSparseBidi (circular 32-token window,
L1 attention, bidirectional aggregation).

Sequence-sharded over 8 cores (512 tokens/core, halo 32).  Per core the
scores use the identity |a-b| = 2*max(a,b) - a - b so the only elementwise
pass is a shifted-window max (DVE, bf16); the width reduction and both
banded aggregations run on the tensor engine; softmax factors
exp(S*(2*Smax - Qs - Ks)) = expmax * u * v are applied as row/column scalings
of the banded weight matrix (v folded into the band, u/den folded into the
vb operand and the forward output).
"""
import math
import numpy as np

BS, NTOK, NHEADS, WIDTH, WIN = 2, 4096, 8, 32, 32
NCORES = 8
TPC = NTOK // NCORES            # 512 tokens per core
G = 2                           # head groups of 4 heads
HL = 4                          # heads per group
F = NHEADS * WIDTH              # 256
RA = TPC + 32                   # 544 attn rows per core (halo 32 on the left)
KS = TPC + 64                   # 576 k rows
SCALE = -1.0 / math.sqrt(WIDTH)
NT = TPC // 128                 # 4 output tiles of 128 tokens

_BASS_CACHE = {}
_PATCHED = False


def _patch_tile_drain():
    """walrus in this container rejects Drain instructions with >=3 sem waits;
    split the TileContext exit drain into one drain per wait."""
    global _PATCHED
    if _PATCHED:
        return
    _PATCHED = True
    import concourse.tile as tile
    import concourse.mybir as mybir

    def _drain_and_barrier(self, tick_clock, wait_clock):
        drain_inst = self.nc.sync.drain()
        wait_clock.add_sem_waits(
            drain_inst.ins, tile.ScopedClock({None: tick_clock.global_clock})
        )
        si = drain_inst.ins.sync_info
        if si is not None and si.on_wait and len(si.on_wait) > 1:
            waits = list(si.on_wait)
            upds = list(si.on_update)
            drain_inst.ins.sync_info = mybir.SyncInfo(
                on_wait=waits[:1], on_update=upds)
            for k in range(1, len(waits)):
                extra = self.nc.sync.drain()
                extra.ins.sync_info = mybir.SyncInfo(
                    on_wait=waits[k:k + 1], on_update=[])
        self.nc.all_engine_barrier()
        popped = self.nc._tile_sem_poison_stack.pop()
        assert popped is self._sem_poison
        self.nc.clear_and_free_semaphores(list(self.sems.allocated().values()))
        self.nc.all_engine_barrier()

    tile.TileContext._drain_and_barrier = _drain_and_barrier


def _build_program(use_softmax):
    _patch_tile_drain()
    import concourse.bass as bass
    import concourse.mybir as mybir
    import concourse.tile as tile
    from concourse.ap import AP

    f32 = mybir.dt.float32
    bf16 = mybir.dt.bfloat16
    Alu = mybir.AluOpType
    Act = mybir.ActivationFunctionType

    # fractional-stride band APs trip the race detector's
    # conservative footprints (false positives on disjoint tensors)
    JB = 4                      # j-block size for the max pass
    NB = 4 * NT                 # 16 (t, hl) band blocks
    EW = 640                    # e_src / b_big width (5 x 128)

    nc = bass.Bass(detect_race_conditions=False)
    qT_d = nc.dram_tensor("qT", [BS, G, 128, RA], bf16, kind="ExternalInput")
    kT_d = nc.dram_tensor("kT", [BS, G, 128, KS], bf16, kind="ExternalInput")
    vf_d = nc.dram_tensor("vfN", [BS, RA, F], bf16, kind="ExternalInput")
    vb_d = nc.dram_tensor("vbN", [BS, RA, F], bf16, kind="ExternalInput")
    out_d = nc.dram_tensor("out", [BS, TPC, F], f32, kind="ExternalOutput")
    # DRAM band images (diagonal writes are only legal on the DRAM side).
    # ExternalInput + host-zeroed: in-program zero fills of Internal DRAM are
    # not dependency-tracked, so a cold first run could read garbage.
    ximg = [nc.dram_tensor(f"ximg{i}", [128, 2560], bf16, kind="ExternalInput")
            for i in range(4)]
    fimg = [nc.dram_tensor(f"fimg{i}", [128, 2560], bf16, kind="ExternalInput")
            for i in range(4)]
    f4img = [nc.dram_tensor(f"f4img{i}", [32, 3072], bf16, kind="ExternalInput")
             for i in range(4)]
    simg = [nc.dram_tensor(f"simg{i}", [32, 2560], bf16, kind="ExternalInput")
            for i in range(4)]
    esimg = [nc.dram_tensor(f"esimg{i}", [128, EW], bf16, kind="ExternalInput")
             for i in range(4)]

    with tile.TileContext(nc) as tc:
        with tc.tile_pool(name="const", bufs=1) as cpool, \
             tc.tile_pool(name="qk", bufs=3) as qk_pool, \
             tc.tile_pool(name="mx", bufs=6) as mx_pool, \
             tc.tile_pool(name="sc", bufs=4) as sc_pool, \
             tc.tile_pool(name="at", bufs=4) as at_pool, \
             tc.tile_pool(name="vv", bufs=4) as vv_pool, \
             tc.tile_pool(name="oo", bufs=3) as oo_pool, \
             tc.tile_pool(name="bd", bufs=2) as bd_pool, \
             tc.tile_pool(name="pw", bufs=2, space="PSUM") as pw_pool, \
             tc.tile_pool(name="pk", bufs=1, space="PSUM") as pk_pool, \
             tc.tile_pool(name="po", bufs=2, space="PSUM") as po_pool:

            # ---- constants -----------------------------------------------
            # BASE_PAD[p, 32 + (p//32)*32] = 2.0 ; lhsT_j = base[:, 32-j:160-j]
            base = cpool.tile([128, 160], bf16, tag="base")
            nc.vector.memset(base[:], 0.0)
            for hl in range(HL):
                nc.vector.memset(base[hl * 32:(hl + 1) * 32,
                                      32 + hl * 32:33 + hl * 32], 2.0)
            s4 = cpool.tile([128, HL], bf16, tag="s4")
            nc.vector.memset(s4[:], 0.0)
            for hl in range(HL):
                nc.vector.memset(s4[hl * 32:(hl + 1) * 32, hl:hl + 1], 1.0)

            # e2 ring stays in SBUF (rect writes only); band images live
            # in DRAM (zero-initialized once, band cells rewritten per use)
            RN = 4
            e2r = []
            for i in range(RN):
                t = cpool.tile([128, 576], bf16, tag=f"e2r{i}", name=f"e2r{i}")
                nc.scalar.memzero(t[:])
                e2r.append(t)
            ctx = {}   # (b, g) -> dict of per-group tiles

            for b in range(BS):
                for g in range(G):
                    rix = (b * G + g) % RN
                    e2 = e2r[rix]

                    # ---- load q/k (feature-major bf16) -------------------
                    q_s = qk_pool.tile([128, RA], bf16, tag="q_s")
                    k_s = qk_pool.tile([128, KS], bf16, tag="k_s")
                    nc.sync.dma_start(q_s[:], qT_d[b, g])
                    nc.sync.dma_start(k_s[:], kT_d[b, g])

                    # ---- Ks -> v = exp(-S*Ks) [4, 576] -------------------
                    pks_a = pk_pool.tile([HL, 512], f32, tag="pks_a")
                    pmix = pk_pool.tile([128, 512], f32, tag="pmix")
                    pks_b = pmix[0:4, 64:128]
                    nc.tensor.matmul(pks_a[:], s4[:], k_s[:, 0:512],
                                     start=True, stop=True)
                    nc.tensor.matmul(pks_b, s4[:], k_s[:, 512:KS],
                                     start=True, stop=True)
                    v_t = sc_pool.tile([HL, KS], bf16, tag="v_t")
                    nc.scalar.activation(v_t[:, 0:512], pks_a[:], Act.Exp,
                                         scale=-SCALE)
                    nc.scalar.activation(v_t[:, 512:KS], pks_b, Act.Exp,
                                         scale=-SCALE)

                    # ---- Qs (transposed) -> uT = exp(-S*Qs) [128, 20] ----
                    puT = pmix[:, 0:4 * (1 + NT)]
                    nc.tensor.matmul(puT[:, 0:4], q_s[:, 0:128], s4[:],
                                     start=True, stop=True)
                    for k in range(NT):
                        nc.tensor.matmul(
                            puT[:, 4 * (k + 1):4 * (k + 2)],
                            q_s[:, 32 + 128 * k:160 + 128 * k], s4[:],
                            start=True, stop=True)
                    uT = sc_pool.tile([128, 4 * (1 + NT)], bf16, tag="uT")
                    nc.scalar.activation(uT[:], puT, Act.Exp, scale=-SCALE)

                    # ---- max pass + score matmuls ------------------------
                    pww_a = pw_pool.tile([128, 512], f32, tag="pww_a")
                    pww_b = pw_pool.tile([128, 32], f32, tag="pww_b")
                    for jb in range(0, WIN, JB):
                        mxb = mx_pool.tile([128, JB * RA], bf16, tag="mxb")
                        in0 = AP(q_s[:].tensor, q_s[:].offset,
                                 [[RA, 128], [0, JB], [1, RA]])
                        in1 = AP(k_s[:].tensor, k_s[:].offset + jb,
                                 [[KS, 128], [1, JB], [1, RA]])
                        o3 = AP(mxb[:].tensor, mxb[:].offset,
                                [[JB * RA, 128], [RA, JB], [1, RA]])
                        nc.vector.tensor_tensor(o3, in0, in1, op=Alu.max)
                        for jl in range(JB):
                            j = jb + jl
                            lhs = base[:, 32 - j:160 - j]
                            nc.tensor.matmul(
                                pww_a[:], lhs, mxb[:, jl * RA:jl * RA + 512],
                                start=(j == 0), stop=(j == WIN - 1))
                            nc.tensor.matmul(
                                pww_b[:], lhs, mxb[:, jl * RA + 512:(jl + 1) * RA],
                                start=(j == 0), stop=(j == WIN - 1))

                    # ---- e = exp(S * 2max-sum) ; e2 = e * v_shift --------
                    e_t = at_pool.tile([128, RA], bf16, tag="e_t")
                    nc.scalar.activation(e_t[:, 0:512], pww_a[:], Act.Exp,
                                         scale=SCALE)
                    nc.scalar.activation(e_t[:, 512:RA], pww_b[:], Act.Exp,
                                         scale=SCALE)
                    vsh = at_pool.tile([128, RA], bf16, tag="vsh")
                    for hl in range(HL):
                        src = AP(v_t[:].tensor, v_t[:].offset + hl * KS,
                                 [[KS, 1], [1, 32], [1, RA]])
                        dst = AP(vsh[:].tensor, vsh[:].offset + hl * 32 * RA,
                                 [[RA, 32], [1, RA]])
                        nc.gpsimd.dma_start(dst, src)
                    nc.vector.tensor_tensor(e2[:, 0:RA], e_t[:], vsh[:],
                                            op=Alu.mult)

                    # ---- A_T (dst-major) transposes into at_big ----------
                    at_big = at_pool.tile([128, EW], bf16, tag="at_big")
                    nc.sync.dma_start_transpose(at_big[:, 0:128], e2[:, 0:128])
                    for k in range(NT):
                        nc.sync.dma_start_transpose(
                            at_big[:, 128 * (k + 1):128 * (k + 2)],
                            e2[:, 32 + 128 * k:160 + 128 * k])

                    # ---- E_src (src-major, shifted) + B + B-reversed -----
                    # E_src[(h,j), sc+j] = e2[(h,j), 32+sc]: diagonal write
                    # into the DRAM image; xbar-transpose reads DRAM directly
                    es_d = esimg[rix]
                    for hl in range(HL):
                        srcE = AP(e2[:].tensor,
                                  e2[:].offset + hl * 32 * 576 + 32,
                                  [[576, 32], [1, RA]])
                        dstE = AP(es_d[0].tensor, es_d[0].offset + hl * 32 * EW,
                                  [[EW + 1, 32], [1, RA]])
                        nc.gpsimd.dma_start(dstE, srcE)
                    b_big = at_pool.tile([128, EW], bf16, tag="b_big")
                    for k in range(5):
                        nc.scalar.dma_start_transpose(
                            b_big[:, 128 * k:128 * (k + 1)],
                            es_d[0:128, 128 * k:128 * (k + 1)])
                    br_big = at_pool.tile([128, EW], bf16, tag="br_big")
                    inR = AP(b_big[:].tensor, b_big[:].offset + 31,
                             [[EW, 128], [32, 20], [-1, 32]])
                    outR = AP(br_big[:].tensor, br_big[:].offset,
                              [[EW, 128], [32, 20], [1, 32]])
                    nc.scalar.activation(outR, inR, Act.Copy)

                    # ---- den: rowsums over j, uden = uT/(1 + uT*rs) ------
                    if use_softmax:
                        rs = sc_pool.tile([128, 4 * (1 + NT)], f32, tag="rs")
                        nc.vector.tensor_reduce(
                            rs[:].rearrange("p (k h o) -> p k h o", h=HL, o=1),
                            at_big[:].rearrange("p (k h j) -> p k h j",
                                                h=HL, j=32),
                            axis=mybir.AxisListType.X, op=Alu.add)
                        den = sc_pool.tile([128, 4 * (1 + NT)], f32, tag="den")
                        nc.vector.tensor_tensor(den[:], rs[:], uT[:], op=Alu.mult)
                        nc.vector.tensor_scalar_add(den[:], den[:], 1.0)
                        rden = sc_pool.tile([128, 4 * (1 + NT)], f32, tag="rden")
                        nc.vector.reciprocal(rden[:], den[:])
                        uden = sc_pool.tile([128, 4 * (1 + NT)], bf16, tag="uden")
                        nc.vector.tensor_tensor(uden[:], uT[:], rden[:],
                                                op=Alu.mult)
                    else:
                        uden = uT

                    # ---- band builds: diag DMA to DRAM image + rect read-back
                    x_d, f_d, f4_d, s_d = (ximg[rix], fimg[rix],
                                           f4img[rix], simg[rix])
                    srcX = AP(at_big[:].tensor, at_big[:].offset + 128,
                              [[EW, 128], [32, NB], [1, 32]])
                    dstX = AP(x_d[0].tensor, x_d[0].offset,
                              [[NB * 160 + 1, 128], [160, NB], [1, 32]])
                    nc.gpsimd.dma_start(dstX, srcX)
                    srcF = AP(br_big[:].tensor, br_big[:].offset,
                              [[EW, 128], [32, NB], [1, 32]])
                    dstF = AP(f_d[0].tensor, f_d[0].offset + 1,
                              [[NB * 160 + 1, 128], [160, NB], [1, 32]])
                    nc.gpsimd.dma_start(dstF, srcF)
                    srcF4 = AP(br_big[:].tensor, br_big[:].offset + 128,
                               [[EW, 32], [32, NB], [1, 32]])
                    dstF4 = AP(f4_d[0].tensor, f4_d[0].offset + 129,
                               [[NB * 192 + 1, 32], [192, NB], [1, 32]])
                    nc.gpsimd.dma_start(dstF4, srcF4)
                    srcS0 = AP(at_big[:].tensor, at_big[:].offset,
                               [[EW, 32], [32, HL], [1, 32]])
                    dstS0 = AP(s_d[0].tensor, s_d[0].offset,
                               [[NB * 160 + 1, 32], [160, HL], [1, 32]])
                    nc.gpsimd.dma_start(dstS0, srcS0)
                    srcS = AP(at_big[:].tensor, at_big[:].offset + 96 * EW + 128,
                              [[EW, 32], [32, NB - HL], [1, 32]])
                    dstS = AP(s_d[0].tensor, s_d[0].offset + HL * 160,
                              [[NB * 160 + 1, 32], [160, NB - HL], [1, 32]])
                    nc.gpsimd.dma_start(dstS, srcS)
                    # rectangular read-backs (only the lhsT columns)
                    x_sb = bd_pool.tile([128, NB * 128], bf16, tag="x_sb")
                    for hv in range(2):
                        nc.scalar.dma_start(
                            AP(x_sb[:].tensor, x_sb[:].offset + hv * 8 * 128,
                               [[NB * 128, 128], [128, 8], [1, 128]]),
                            AP(x_d[0].tensor, x_d[0].offset + hv * 8 * 160,
                               [[2560, 128], [160, 8], [1, 128]]))
                    f_sb = bd_pool.tile([128, NB * 128], bf16, tag="f_sb")
                    for hv in range(2):
                        nc.scalar.dma_start(
                            AP(f_sb[:].tensor, f_sb[:].offset + hv * 8 * 128,
                               [[NB * 128, 128], [128, 8], [1, 128]]),
                            AP(f_d[0].tensor, f_d[0].offset + 32 + hv * 8 * 160,
                               [[2560, 128], [160, 8], [1, 128]]))
                    f4_sb = bd_pool.tile([32, NB * 128], bf16, tag="f4_sb")
                    nc.scalar.dma_start(
                        AP(f4_sb[:].tensor, f4_sb[:].offset,
                           [[NB * 128, 32], [128, NB], [1, 128]]),
                        AP(f4_d[0].tensor, f4_d[0].offset + 32,
                           [[3072, 32], [192, NB], [1, 128]]))
                    s_sb = bd_pool.tile([32, NB * 128], bf16, tag="s_sb")
                    for hv in range(2):
                        nc.scalar.dma_start(
                            AP(s_sb[:].tensor, s_sb[:].offset + hv * 8 * 128,
                               [[NB * 128, 32], [128, 8], [1, 128]]),
                            AP(s_d[0].tensor, s_d[0].offset + 32 + hv * 8 * 160,
                               [[2560, 32], [160, 8], [1, 128]]))

                    # ---- vf / vb loads; vb scaled by uden ----------------
                    vf_big = vv_pool.tile([128, 4 * 128], bf16, tag="vf_big")
                    srcVF = AP(vf_d[0].tensor,
                               vf_d[b].offset + g * 128,
                               [[F, 128], [128 * F, NT], [1, 128]])
                    dstVF = AP(vf_big[:].tensor, vf_big[:].offset,
                               [[512, 128], [128, NT], [1, 128]])
                    nc.sync.dma_start(dstVF, srcVF)
                    vf_x = vv_pool.tile([32, 128], bf16, tag="vf_x")
                    nc.sync.dma_start(vf_x[:], vf_d[b, 512:RA,
                                                    g * 128:(g + 1) * 128])
                    vb_big = vv_pool.tile([128, 4 * 128], bf16, tag="vb_big")
                    srcVB = AP(vb_d[0].tensor,
                               vb_d[b].offset + 32 * F + g * 128,
                               [[F, 128], [128 * F, NT], [1, 128]])
                    dstVB = AP(vb_big[:].tensor, vb_big[:].offset,
                               [[512, 128], [128, NT], [1, 128]])
                    nc.sync.dma_start(dstVB, srcVB)
                    vb_pre = vv_pool.tile([32, 128], bf16, tag="vb_pre")
                    nc.sync.dma_start(vb_pre[:], vb_d[b, 0:32,
                                                      g * 128:(g + 1) * 128])
                    vbs_big = vv_pool.tile([128, 4 * 128], bf16, tag="vbs_big")
                    uIn = AP(uden[:].tensor, uden[:].offset + 4,
                             [[4 * (1 + NT), 128], [4, NT], [1, HL], [0, 32]])
                    nc.vector.tensor_tensor(
                        vbs_big[:].rearrange("p (k h w) -> p k h w", h=HL, w=32),
                        vb_big[:].rearrange("p (k h w) -> p k h w", h=HL, w=32),
                        uIn, op=Alu.mult)
                    vbs_pre = vv_pool.tile([32, 128], bf16, tag="vbs_pre")
                    uPre = AP(uden[:].tensor, uden[:].offset,
                              [[4 * (1 + NT), 32], [1, HL], [0, 32]])
                    nc.vector.tensor_tensor(
                        vbs_pre[:].rearrange("p (h w) -> p h w", w=32),
                        vb_pre[:].rearrange("p (h w) -> p h w", w=32),
                        uPre, op=Alu.mult)
                    strips = [vbs_pre]
                    for t in range(1, NT):
                        st_k = vv_pool.tile([32, 128], bf16, tag=f"vst{t}")
                        nc.gpsimd.dma_start(st_k[:],
                                            vbs_big[96:128, 128 * (t - 1):128 * t])
                        strips.append(st_k)

                    ctx[(b, g)] = dict(uden=uden, x_sb=x_sb, f_sb=f_sb,
                                       f4_sb=f4_sb, s_sb=s_sb, vf_big=vf_big,
                                       vf_x=vf_x, vbs_big=vbs_big,
                                       strips=strips)

                # ---- aggregation per (b, t), all 8 heads -----------------
                for t in range(NT):
                    pfb = po_pool.tile([128, 2 * F], f32, tag="pfb")
                    for g in range(G):
                        cc = ctx[(b, g)]
                        for hl in range(HL):
                            h = g * HL + hl
                            blk = 4 * t + hl
                            cs = slice(h * 32, (h + 1) * 32)
                            csb = slice(F + h * 32, F + (h + 1) * 32)
                            xl = cc["x_sb"][:, blk * 128:(blk + 1) * 128]
                            fl = cc["f_sb"][:, blk * 128:(blk + 1) * 128]
                            f4l = cc["f4_sb"][:, blk * 128:(blk + 1) * 128]
                            sl = cc["s_sb"][:, blk * 128:(blk + 1) * 128]
                            vfs1 = cc["vf_big"][:, 128 * t + hl * 32:
                                                128 * t + hl * 32 + 32]
                            if t < NT - 1:
                                vfs2 = cc["vf_big"][0:32,
                                                    128 * (t + 1) + hl * 32:
                                                    128 * (t + 1) + hl * 32 + 32]
                            else:
                                vfs2 = cc["vf_x"][:, hl * 32:hl * 32 + 32]
                            nc.tensor.matmul(pfb[:, cs], fl, vfs1,
                                             start=True, stop=False)
                            nc.tensor.matmul(pfb[:, cs], f4l, vfs2,
                                             start=False, stop=True)
                            nc.tensor.matmul(
                                pfb[:, csb], sl,
                                cc["strips"][t][:, hl * 32:hl * 32 + 32],
                                start=True, stop=False)
                            nc.tensor.matmul(
                                pfb[:, csb], xl,
                                cc["vbs_big"][:, 128 * t + hl * 32:
                                              128 * t + hl * 32 + 32],
                                start=False, stop=True)
                    # combine: out = uden_bcast * pf + pb
                    o_s = oo_pool.tile([128, F], f32, tag="o_s")
                    for g in range(G):
                        uden = ctx[(b, g)]["uden"]
                        gs = slice(g * 128, (g + 1) * 128)
                        u_bt = AP(uden[:].tensor, uden[:].offset + 4 * (t + 1),
                                  [[4 * (1 + NT), 128], [1, 4], [0, 32]])
                        nc.vector.tensor_tensor(
                            o_s[:, gs].rearrange("p (h w) -> p h w", w=32),
                            pfb[:, g * 128:(g + 1) * 128].rearrange(
                                "p (h w) -> p h w", w=32),
                            u_bt, op=Alu.mult)
                        nc.vector.tensor_tensor(o_s[:, gs], o_s[:, gs],
                                                pfb[:, F + g * 128:F + (g + 1) * 128],
                                                op=Alu.add)
                    nc.gpsimd.dma_start(out_d[b, 128 * t:128 * (t + 1), :], o_s[:])
    _split_excess_waits(nc, mybir)
    return nc


def _split_excess_waits(nc, mybir, maxw=1):
    """This container's walrus allows fewer sem waits per instruction than
    bass assumes; hoist excess waits into same-engine no-ops (the engine
    sequencer executes them in order, so semantics are unchanged)."""
    cnt = 0
    for fn in nc.m.functions:
        for bb in fn.blocks:
            new = []
            for ins in bb.instructions:
                si = ins.sync_info
                if si is not None and si.on_wait and len(si.on_wait) > maxw:
                    waits = list(si.on_wait)
                    ups = list(si.on_update)
                    for w in waits[:-maxw]:
                        cnt += 1
                        nop = mybir.InstNoOp(
                            name=f"{ins.name}-hw{cnt}", ins=[], outs=[])
                        nop.engine = ins.engine
                        nop.sync_info = mybir.SyncInfo(on_wait=[w], on_update=[])
                        new.append(nop)
                    ins.sync_info = mybir.SyncInfo(
                        on_wait=waits[-maxw:], on_update=ups)
                new.append(ins)
            bb.instructions = new


# ------------------------------------------------------------------ host side
def _prep_core_inputs(q, k, vf, vb, core):
    """Build the per-core input map (bf16, pre-transposed q/k)."""
    import ml_dtypes
    bf = ml_dtypes.bfloat16
    T0 = core * TPC
    ra_rows = (T0 - 32 + np.arange(RA)) % NTOK
    ks_rows = (T0 - 32 + np.arange(KS)) % NTOK
    vf_rows = (T0 + np.arange(RA)) % NTOK
    vb_rows = (T0 - 32 + np.arange(RA)) % NTOK
    # q/k feature-major: [BS, G, 128(p=hl*32+w), rows]
    qT = np.ascontiguousarray(
        q[:, ra_rows].reshape(BS, RA, G, 128).transpose(0, 2, 3, 1)).astype(bf)
    kT = np.ascontiguousarray(
        k[:, ks_rows].reshape(BS, KS, G, 128).transpose(0, 2, 3, 1)).astype(bf)
    vfN = np.ascontiguousarray(vf[:, vf_rows].reshape(BS, RA, F)).astype(bf)
    vbN = np.ascontiguousarray(vb[:, vb_rows].reshape(BS, RA, F)).astype(bf)
    m = {"qT": qT, "kT": kT, "vfN": vfN, "vbN": vbN}
    for i in range(4):
        m[f"ximg{i}"] = np.zeros((128, 2560), bf)
        m[f"fimg{i}"] = np.zeros((128, 2560), bf)
        m[f"f4img{i}"] = np.zeros((32, 3072), bf)
        m[f"simg{i}"] = np.zeros((32, 2560), bf)
        m[f"esimg{i}"] = np.zeros((128, 640), bf)
    return m


def _run_bass(q, k, vf, vb, use_softmax):
    from concourse.bass_utils import run_bass_kernel_spmd

    key = bool(use_softmax)
    if key not in _BASS_CACHE:
        _BASS_CACHE[key] = _build_program(key)
    nc = _BASS_CACHE[key]
    in_maps = [_prep_core_inputs(q, k, vf, vb, c) for c in range(NCORES)]
    res = run_bass_kernel_spmd(nc, in_maps, core_ids=list(range(NCORES)))
    out = np.empty((BS, NTOK, F), np.float32)
    for c in range(NCORES):
        out[:, c * TPC:(c + 1) * TPC] = res.results[c]["out"].reshape(BS, TPC, F)
    return out.reshape(BS, NTOK, NHEADS, WIDTH)


# ---------------------------------------------------------------- numpy path
def _numpy_full(vf, vb, q, k, coo, use_softmax):
    dst = coo[:, 0].astype(np.int64)
    src = coo[:, 1].astype(np.int64)
    ww = SCALE * np.abs(q[:, dst] - k[:, src]).sum(-1).transpose(1, 0, 2)
    if use_softmax:
        e = np.exp(ww)
        denom = np.zeros((NTOK,) + ww.shape[1:], np.float32)
        np.add.at(denom, dst, e)
        denom += 1.0
        attn = e / denom[dst]
    else:
        attn = np.exp(ww)
    vfo = np.zeros((NTOK, BS, NHEADS, WIDTH), np.float32)
    np.add.at(vfo, dst, attn[..., None] * vf[:, src].transpose(1, 0, 2, 3))
    vbo = np.zeros((NTOK, BS, NHEADS, WIDTH), np.float32)
    np.add.at(vbo, src, attn[..., None] * vb[:, dst].transpose(1, 0, 2, 3))
    return (vfo + vbo).transpose(1, 0, 2, 3).astype(np.float32)


def kernel(**inputs):
    q = np.asarray(inputs["q"], np.float32)
    k = np.asarray(inputs["k"], np.float32)
    vf = np.asarray(inputs["vf"], np.float32)
    vb = np.asarray(inputs["vb"], np.float32)
    coo = np.asarray(inputs["coo"])
    use_softmax = int(np.asarray(inputs.get("use_softmax", 1)))

    d = np.repeat(np.arange(NTOK), WIN)
    s = (d + np.tile(np.arange(WIN), NTOK)) % NTOK
    circular = coo.shape == (NTOK * WIN, 4) and \
        np.array_equal(coo[:, 0], d) and np.array_equal(coo[:, 1], s)
    if circular:
        try:
            return _run_bass(q, k, vf, vb, use_softmax)
        except Exception:
            import traceback
            traceback.print_exc()
    return _numpy_full(vf, vb, q, k, coo, use_softmax)

